# revision 32
# baseline (speedup 1.0000x reference)
"""Trainium2 Bass kernel for nn_PixelAggregationNetwork.

Strategy (8 NeuronCores, memory-bound):
  x is [B=4, C=32, H=512, W=500] f32 (~131 MB). All downstream math
  (tree/LCA/loss) operates on tiny per-segment reductions of x, so the
  kernel's only real job is one streaming pass over x.

  Shard along H: core k owns rows [64k, 64k+64) for all (b, c), viewed as
  [B*C = 128 partitions, 64*500]. Input streams in 4-row chunks over THREE
  DMA paths concurrently (to saturate the ~358 GB/s per-core HBM port):
    - Pool/SWDGE: 8 chunks, casting f32->f16 in the DMA itself
    - SP-HWDGE: 4 chunks raw f32 (Pool casts them to f16 afterwards)
    - ACT-HWDGE: 4 chunks raw f32 (ACT casts them)
  Compute per chunk:
    - VectorE tensor_reduce -> per-(row,strip) sums, folded to [128, 10]
      at the end (f32 chunks are reduced straight from the f32 staging)
    - TensorE: per row, matmul with stationary [128, 4] channel-mean mask,
      moving = the row's 500 pixels f16 -> gray [4, 500] in PSUM. The 4
      rows of a chunk pack into ONE PSUM bank at base partitions
      0/32/64/96; 16 chunks over 8 banks (each bank reused once, gated by
      a standalone wait on the evacuation semaphore).
    - ACT evacuates each bank [128, 500] f32->f16 and finally writes gray
      out as 4 row-major DMAs (one per batch, partition-strided source).
  Core outputs: segment sums [128, 10] f32, gray [4, 64*500] f16.
  Host combines partials (f64) and finishes the 21-node hierarchy + loss.

  Raw Bass (no TileContext): the walrus build here supports only ONE
  embedded sync-wait per DMA/matmul/drain instruction, which Tile's
  auto-generated semaphores and kernel-tail drain violate structurally.
  With explicit semaphores, standalone single-wait EVENT_SEMAPHORE
  instructions express every multi-dependency legally.
"""

import numpy as np
from contextlib import ExitStack

import concourse.bass as bass
import concourse.mybir as mybir
from concourse.bass_utils import run_bass_kernel_spmd

B, C, H, W = 4, 32, 512, 500
S = 10
SW = W // S
TW = 0.5
MARGIN = 1.0
REG_W = 0.01

NCORES = 8
HC = H // NCORES          # 64 rows per core
R = 4                     # rows per chunk == rows per PSUM bank
NCH = HC // R             # 16 chunks per core
CW = R * W                # chunk free width (2000)
NBANK = 8                 # physical PSUM banks used

F32 = mybir.dt.float32
F16 = mybir.dt.float16

# chunk -> DMA path: even chunks on Pool/SWDGE (casting), odd chunks
# alternate SP / ACT (f32; Pool casts SP's, ACT casts its own).
def _path(ch):
    if ch % 2 == 0:
        return "pool"
    return "sp" if ch % 4 == 1 else "act"

PATHS = [_path(ch) for ch in range(NCH)]
ARRIVE = []
_cnt = {"pool": 0, "sp": 0, "act": 0}
for _ch in range(NCH):
    _cnt[PATHS[_ch]] += 1
    ARRIVE.append(_cnt[PATHS[_ch]])
F32SLOT = {}
for _ch in range(NCH):
    if PATHS[_ch] != "pool":
        F32SLOT[_ch] = len(F32SLOT)
NF32 = len(F32SLOT)


# ---------------------------------------------------------------- tree/LCA
def _build_tree():
    sizes = []
    n = S
    while True:
        sizes.append(n)
        if n == 1:
            break
        n = (n + 1) // 2
    offs = np.cumsum([0] + sizes)
    total = int(offs[-1])
    parent = np.arange(total)
    level = np.zeros(total, np.int32)
    for l, sz in enumerate(sizes):
        for i in range(sz):
            g = offs[l] + i
            level[g] = l
            if l + 1 < len(sizes):
                parent[g] = offs[l + 1] + i // 2
    L = len(sizes)
    chain = np.zeros((total, L), np.int64)
    for g in range(total):
        for l in range(L):
            if l < level[g]:
                chain[g, l] = -1 - g
            else:
                a = g
                while level[a] < l:
                    a = int(parent[a])
                chain[g, l] = a
    return sizes, parent.astype(np.int32), level, chain


SIZES, PARENT, LEVEL, CHAIN = _build_tree()
MAXL = len(SIZES) - 1
NTOT = PARENT.shape[0]


def _lca_matrix():
    eq = CHAIN[:, None, :] == CHAIN[None, :, :]
    first = np.argmax(eq, axis=-1)
    return CHAIN[np.arange(NTOT)[:, None], first].astype(np.int32)


LCA = _lca_matrix()


# ---------------------------------------------------------------- device program
_PROGRAM = None


def _build_program():
    nc = bass.Bass(trn_type="TRN2", num_swdge_queues=4)
    xs = nc.declare_dram_parameter("xs", [B * C, HC * W], F32, isOutput=False)
    mask = nc.declare_dram_parameter("mask", [B * C, B], F16, isOutput=False)
    seg_out = nc.declare_dram_parameter("seg_out", [B * C, S], F32, isOutput=True)
    gray_out = nc.declare_dram_parameter("gray_out", [B, HC * W], F16, isOutput=True)

    with ExitStack() as ctx:
        t16 = ctx.enter_context(nc.sbuf_tensor([B * C, HC * W], F16))
        tf32 = ctx.enter_context(nc.sbuf_tensor([B * C, NF32 * CW], F32))
        mask_t = ctx.enter_context(nc.sbuf_tensor([B * C, B], F16))
        segbuf = ctx.enter_context(nc.sbuf_tensor([B * C, NCH * R * S], F32))
        seg_final = ctx.enter_context(nc.sbuf_tensor([B * C, S], F32))
        # gbuf[32q+b, (ch, w)] = gray[b, (ch*R + q)*W + w]
        gbuf = ctx.enter_context(nc.sbuf_tensor([B * C, NCH * W], F16))
        psum = [ctx.enter_context(nc.psum_tensor(f"psb{i}", [B * C, W], F32))
                for i in range(NBANK)]
        a_sem = ctx.enter_context(nc.semaphore("a_sem"))    # pool in-DMAs
        b_sem = ctx.enter_context(nc.semaphore("b_sem"))    # sp in-DMAs
        c_sem = ctx.enter_context(nc.semaphore("c_sem"))    # act in-DMAs
        pc_sem = ctx.enter_context(nc.semaphore("pc_sem"))  # pool casts
        ac_sem = ctx.enter_context(nc.semaphore("ac_sem"))  # act casts
        pe_sem = ctx.enter_context(nc.semaphore("pe_sem"))
        ev_sem = ctx.enter_context(nc.semaphore("ev_sem"))  # evacuations
        dve_sem = ctx.enter_context(nc.semaphore("dve_sem"))
        g_sem = ctx.enter_context(nc.semaphore("g_sem"))
        s_sem = ctx.enter_context(nc.semaphore("s_sem"))

        def tsl(ch):
            return slice(ch * CW, (ch + 1) * CW)

        def fsl(ch):
            s0 = F32SLOT[ch] * CW
            return slice(s0, s0 + CW)

        # ---- input DMA issues ------------------------------------------
        nc.gpsimd.dma_start(mask_t[:], mask[:]).then_inc(a_sem, 16)
        for ch in range(NCH):
            if PATHS[ch] == "pool":
                nc.gpsimd.dma_start(t16[:, tsl(ch)], xs[:, tsl(ch)]) \
                    .then_inc(a_sem, 16)
        for ch in range(NCH):
            if PATHS[ch] == "sp":
                nc.sync.dma_start(tf32[:, fsl(ch)], xs[:, tsl(ch)]) \
                    .then_inc(b_sem, 16)
        for ch in range(NCH):
            if PATHS[ch] == "act":
                nc.scalar.dma_start(tf32[:, fsl(ch)], xs[:, tsl(ch)]) \
                    .then_inc(c_sem, 16)

        # ---- ACT: casts (all f32-staged chunks) interleaved with PSUM
        # evacuations, in chunk order so the PE never stalls on bank reuse.
        # GpSimd must stay DMA-only: SWDGE descriptors are processed by the
        # GpSimd engine itself, so any compute there throttles the Pool path.
        CAST_RANK = {}
        for ch in range(NCH):
            if PATHS[ch] != "pool":
                CAST_RANK[ch] = len(CAST_RANK) + 1

        def emit_cast(ch):
            sem = b_sem if PATHS[ch] == "sp" else c_sem
            nc.scalar.wait_ge(sem, 16 * ARRIVE[ch])
            nc.scalar.copy(t16[:, tsl(ch)], tf32[:, fsl(ch)]) \
                .then_inc(ac_sem, 1)

        def emit_evac(e):
            nc.scalar.wait_ge(pe_sem, e + 1)
            nc.scalar.copy(
                gbuf[:, e * W:(e + 1) * W], psum[e % NBANK][:]
            ).then_inc(ev_sem, 1)

        evacs_done = 0
        for ch in range(NCH):
            if PATHS[ch] != "pool":
                emit_cast(ch)
            if ch >= 4:
                emit_evac(ch - 4)
                evacs_done += 1
        for e in range(evacs_done, NCH):
            emit_evac(e)

        # ---- PE: per row, mask-stationary gray matmul -------------------
        for ch in range(NCH):
            if PATHS[ch] == "pool":
                nc.tensor.wait_ge(a_sem, 16 * (ARRIVE[ch] + 1))
            else:
                nc.tensor.wait_ge(ac_sem, CAST_RANK[ch])
            if ch >= NBANK:          # bank reuse: wait for its evacuation
                nc.tensor.wait_ge(ev_sem, ch - NBANK + 1)
            pst = psum[ch % NBANK]
            for q in range(R):
                row = ch * R + q
                mm = nc.tensor.matmul(
                    pst[32 * q:32 * q + B, :],
                    mask_t[:],
                    t16[:, row * W:(row + 1) * W],
                    start=True, stop=True,
                    skip_group_check=True,
                    tile_position=(0, 32 * q),
                )
            mm.then_inc(pe_sem, 1)

        # ---- ACT: gray out ----------------------------------------------
        for b in range(B):
            # gbuf partitions {b, 32+b, 64+b, 96+b} -> gray_out[b] rows
            src = gbuf[b:b + 97:32, :].rearrange(
                "q (ch w) -> q ch w", ch=NCH, w=W
            )
            dst = gray_out[b, :].rearrange(
                "(ch q w) -> q ch w", ch=NCH, q=R, w=W
            )
            nc.scalar.dma_start(dst, src).then_inc(g_sem, 16)
        nc.scalar.wait_ge(g_sem, 16 * B)

        # ---- DVE: strip sums -------------------------------------------
        for ch in range(NCH):
            if PATHS[ch] == "pool":
                nc.vector.wait_ge(a_sem, 16 * (ARRIVE[ch] + 1))
                src = t16[:, tsl(ch)]
            else:
                sem = b_sem if PATHS[ch] == "sp" else c_sem
                nc.vector.wait_ge(sem, 16 * ARRIVE[ch])
                src = tf32[:, fsl(ch)]
            v = src.rearrange("p (a w) -> p a w", a=R * S, w=SW)
            nc.vector.tensor_reduce(
                segbuf[:, ch * R * S:(ch + 1) * R * S], v,
                axis=mybir.AxisListType.X, op=mybir.AluOpType.add,
            )
        nc.vector.tensor_reduce(
            seg_final[:],
            segbuf[:].rearrange("p (a s) -> p s a", a=NCH * R, s=S),
            axis=mybir.AxisListType.X, op=mybir.AluOpType.add,
        ).then_inc(dve_sem, 1)

        # ---- SP: seg out ------------------------------------------------
        nc.sync.wait_ge(dve_sem, 1)
        nc.sync.dma_start(seg_out[:], seg_final[:]).then_inc(s_sem, 16)
        nc.sync.wait_ge(s_sem, 16)
    return nc


def _get_program():
    global _PROGRAM
    if _PROGRAM is None:
        _PROGRAM = _build_program()
    return _PROGRAM


def _make_mask():
    m = np.zeros((B * C, B), np.float16)
    m[np.arange(B * C), np.arange(B * C) // C] = 1.0 / C
    return m


def _run_device(x, **kwargs):
    nc = _get_program()
    mask_np = _make_mask()
    in_maps = []
    for k in range(NCORES):
        xs = np.ascontiguousarray(
            x[:, :, k * HC:(k + 1) * HC, :].reshape(B * C, HC * W)
        )
        in_maps.append({"xs": xs, "mask": mask_np})
    return run_bass_kernel_spmd(nc, in_maps, list(range(NCORES)), **kwargs)


def _finalize(seg, gray):
    """seg: [B*C, S] f64 total strip sums; gray: [B, H, W] f64 channel means."""
    nodes = (seg / (H * SW)).reshape(B, C, S).transpose(0, 2, 1)      # [B,S,C]
    texture = (gray ** 2).reshape(B, H, S, SW).sum(axis=(1, 3))      # [B,S]
    feats = nodes * (1.0 - TW) + texture[..., None] * TW             # [B,S,C]

    reg = 0.0
    cur = feats
    for sz in SIZES[1:]:
        n = cur.shape[1]
        ids = np.arange(n) // 2
        counts = np.bincount(ids, minlength=sz).astype(np.float64)
        summed = np.zeros((sz, B, C), np.float64)
        np.add.at(summed, ids, cur.transpose(1, 0, 2))
        cur = (summed / counts[:, None, None]).transpose(1, 0, 2)
        reg += (cur ** 2).mean()

    diff = feats[:, :, None, :] - feats[:, None, :, :]
    d = np.sqrt((diff ** 2).sum(-1) + 1e-12)                          # [B,S,S]
    w = 1.0 - LEVEL[LCA[:S, :S]].astype(np.float64) / MAXL
    loss = (w * d ** 2 + (1.0 - w) * np.maximum(MARGIN - d, 0.0) ** 2).mean() \
        + REG_W * reg

    tree = np.broadcast_to(
        np.stack([PARENT, LEVEL], axis=-1).astype(np.int32)[None], (B, NTOT, 2)
    ).copy()
    return tree, np.float32(loss)


def _gather(res):
    seg = np.zeros((B * C, S), np.float64)
    gray = np.zeros((B, H, W), np.float64)
    for k in range(NCORES):
        seg += res.results[k]["seg_out"].astype(np.float64)
        g = res.results[k]["gray_out"].astype(np.float64)      # [B, HC*W]
        gray[:, k * HC:(k + 1) * HC, :] = g.reshape(B, HC, W)
    return seg, gray


def kernel(x):
    x = np.asarray(x, dtype=np.float32)
    res = _run_device(x)
    seg, gray = _gather(res)
    return _finalize(seg, gray)


# revision 33
# speedup vs baseline: 1.1942x; 1.1942x over previous
"""Trainium2 Bass kernel for nn_PixelAggregationNetwork.

Strategy (8 NeuronCores, memory-bound):
  x is [B=4, C=32, H=512, W=500] f32 (~131 MB). All downstream math
  (tree/LCA/loss) operates on tiny per-segment reductions of x, so the
  kernel's only real job is one streaming pass over x.

  Shard along H: core k owns rows [64k, 64k+64) for all (b, c), viewed as
  [B*C = 128 partitions, 64*500]. Input streams in 4-row chunks over THREE
  DMA paths concurrently (to saturate the ~358 GB/s per-core HBM port):
    - Pool/SWDGE: 8 chunks, casting f32->f16 in the DMA itself
    - SP-HWDGE: 4 chunks raw f32 (Pool casts them to f16 afterwards)
    - ACT-HWDGE: 4 chunks raw f32 (ACT casts them)
  Compute per chunk:
    - VectorE tensor_reduce -> per-(row,strip) sums, folded to [128, 10]
      at the end (f32 chunks are reduced straight from the f32 staging)
    - TensorE: per row, matmul with stationary [128, 4] channel-mean mask,
      moving = the row's 500 pixels f16 -> gray [4, 500] in PSUM. The 4
      rows of a chunk pack into ONE PSUM bank at base partitions
      0/32/64/96; 16 chunks over 8 banks (each bank reused once, gated by
      a standalone wait on the evacuation semaphore).
    - ACT evacuates each bank [128, 500] f32->f16 and finally writes gray
      out as 4 row-major DMAs (one per batch, partition-strided source).
  Core outputs: segment sums [128, 10] f32, gray [4, 64*500] f16.
  Host combines partials (f64) and finishes the 21-node hierarchy + loss.

  Raw Bass (no TileContext): the walrus build here supports only ONE
  embedded sync-wait per DMA/matmul/drain instruction, which Tile's
  auto-generated semaphores and kernel-tail drain violate structurally.
  With explicit semaphores, standalone single-wait EVENT_SEMAPHORE
  instructions express every multi-dependency legally.
"""

import numpy as np
from contextlib import ExitStack

import concourse.bass as bass
import concourse.mybir as mybir
from concourse.bass_utils import run_bass_kernel_spmd

B, C, H, W = 4, 32, 512, 500
S = 10
SW = W // S
TW = 0.5
MARGIN = 1.0
REG_W = 0.01

NCORES = 8
HC = H // NCORES          # 64 rows per core
R = 4                     # rows per chunk == rows per PSUM bank
NCH = HC // R             # 16 chunks per core
CW = R * W                # chunk free width (2000)
NBANK = 8                 # physical PSUM banks used

F32 = mybir.dt.float32
F16 = mybir.dt.float16

# chunk -> DMA path: even chunks on Pool/SWDGE (casting), odd chunks
# alternate SP / ACT (f32; Pool casts SP's, ACT casts its own).
def _path(ch):
    return "pool"

PATHS = [_path(ch) for ch in range(NCH)]
ARRIVE = []
_cnt = {"pool": 0, "sp": 0, "act": 0}
for _ch in range(NCH):
    _cnt[PATHS[_ch]] += 1
    ARRIVE.append(_cnt[PATHS[_ch]])
F32SLOT = {}
for _ch in range(NCH):
    if PATHS[_ch] != "pool":
        F32SLOT[_ch] = len(F32SLOT)
NF32 = len(F32SLOT)


# ---------------------------------------------------------------- tree/LCA
def _build_tree():
    sizes = []
    n = S
    while True:
        sizes.append(n)
        if n == 1:
            break
        n = (n + 1) // 2
    offs = np.cumsum([0] + sizes)
    total = int(offs[-1])
    parent = np.arange(total)
    level = np.zeros(total, np.int32)
    for l, sz in enumerate(sizes):
        for i in range(sz):
            g = offs[l] + i
            level[g] = l
            if l + 1 < len(sizes):
                parent[g] = offs[l + 1] + i // 2
    L = len(sizes)
    chain = np.zeros((total, L), np.int64)
    for g in range(total):
        for l in range(L):
            if l < level[g]:
                chain[g, l] = -1 - g
            else:
                a = g
                while level[a] < l:
                    a = int(parent[a])
                chain[g, l] = a
    return sizes, parent.astype(np.int32), level, chain


SIZES, PARENT, LEVEL, CHAIN = _build_tree()
MAXL = len(SIZES) - 1
NTOT = PARENT.shape[0]


def _lca_matrix():
    eq = CHAIN[:, None, :] == CHAIN[None, :, :]
    first = np.argmax(eq, axis=-1)
    return CHAIN[np.arange(NTOT)[:, None], first].astype(np.int32)


LCA = _lca_matrix()


# ---------------------------------------------------------------- device program
_PROGRAM = None


def _build_program():
    nc = bass.Bass(trn_type="TRN2", num_swdge_queues=4)
    xs = nc.declare_dram_parameter("xs", [B * C, HC * W], F32, isOutput=False)
    mask = nc.declare_dram_parameter("mask", [B * C, B], F16, isOutput=False)
    seg_out = nc.declare_dram_parameter("seg_out", [B * C, S], F32, isOutput=True)
    gray_out = nc.declare_dram_parameter("gray_out", [B, HC * W], F16, isOutput=True)

    with ExitStack() as ctx:
        t16 = ctx.enter_context(nc.sbuf_tensor([B * C, HC * W], F16))
        mask_t = ctx.enter_context(nc.sbuf_tensor([B * C, B], F16))
        segbuf = ctx.enter_context(nc.sbuf_tensor([B * C, NCH * R * S], F32))
        seg_final = ctx.enter_context(nc.sbuf_tensor([B * C, S], F32))
        # gbuf[32q+b, (ch, w)] = gray[b, (ch*R + q)*W + w]
        gbuf = ctx.enter_context(nc.sbuf_tensor([B * C, NCH * W], F16))
        psum = [ctx.enter_context(nc.psum_tensor(f"psb{i}", [B * C, W], F32))
                for i in range(NBANK)]
        a_sem = ctx.enter_context(nc.semaphore("a_sem"))    # pool in-DMAs
        b_sem = ctx.enter_context(nc.semaphore("b_sem"))    # sp in-DMAs
        pe_sem = ctx.enter_context(nc.semaphore("pe_sem"))
        ev_sem = ctx.enter_context(nc.semaphore("ev_sem"))  # evacuations
        dve_sem = ctx.enter_context(nc.semaphore("dve_sem"))
        g_sem = ctx.enter_context(nc.semaphore("g_sem"))
        s_sem = ctx.enter_context(nc.semaphore("s_sem"))

        def tsl(ch):
            return slice(ch * CW, (ch + 1) * CW)

        # ---- input DMA issues ------------------------------------------
        nc.sync.dma_start(mask_t[:], mask[:]).then_inc(b_sem, 16)
        for ch in range(NCH):
            nc.gpsimd.dma_start(t16[:, tsl(ch)], xs[:, tsl(ch)]) \
                .then_inc(a_sem, 16)

        # ---- ACT: PSUM evacuations ------------------------------------
        for e in range(NCH):
            nc.scalar.wait_ge(pe_sem, e + 1)
            nc.scalar.copy(
                gbuf[:, e * W:(e + 1) * W], psum[e % NBANK][:]
            ).then_inc(ev_sem, 1)

        # ---- PE: per row, mask-stationary gray matmul -------------------
        nc.tensor.wait_ge(b_sem, 16)     # mask
        for ch in range(NCH):
            nc.tensor.wait_ge(a_sem, 16 * ARRIVE[ch])
            if ch >= NBANK:          # bank reuse: wait for its evacuation
                nc.tensor.wait_ge(ev_sem, ch - NBANK + 1)
            pst = psum[ch % NBANK]
            for q in range(R):
                row = ch * R + q
                mm = nc.tensor.matmul(
                    pst[32 * q:32 * q + B, :],
                    mask_t[:],
                    t16[:, row * W:(row + 1) * W],
                    start=True, stop=True,
                    skip_group_check=True,
                    tile_position=(0, 32 * q),
                )
            mm.then_inc(pe_sem, 1)

        # ---- ACT: gray out ----------------------------------------------
        for b in range(B):
            # gbuf partitions {b, 32+b, 64+b, 96+b} -> gray_out[b] rows
            src = gbuf[b:b + 97:32, :].rearrange(
                "q (ch w) -> q ch w", ch=NCH, w=W
            )
            dst = gray_out[b, :].rearrange(
                "(ch q w) -> q ch w", ch=NCH, q=R, w=W
            )
            nc.scalar.dma_start(dst, src).then_inc(g_sem, 16)
        nc.scalar.wait_ge(g_sem, 16 * B)

        # ---- DVE: strip sums -------------------------------------------
        for ch in range(NCH):
            nc.vector.wait_ge(a_sem, 16 * ARRIVE[ch])
            v = t16[:, tsl(ch)].rearrange("p (a w) -> p a w", a=R * S, w=SW)
            nc.vector.tensor_reduce(
                segbuf[:, ch * R * S:(ch + 1) * R * S], v,
                axis=mybir.AxisListType.X, op=mybir.AluOpType.add,
            )
        nc.vector.tensor_reduce(
            seg_final[:],
            segbuf[:].rearrange("p (a s) -> p s a", a=NCH * R, s=S),
            axis=mybir.AxisListType.X, op=mybir.AluOpType.add,
        ).then_inc(dve_sem, 1)

        # ---- SP: seg out ------------------------------------------------
        nc.sync.wait_ge(dve_sem, 1)
        nc.sync.dma_start(seg_out[:], seg_final[:]).then_inc(s_sem, 16)
        nc.sync.wait_ge(s_sem, 16)
    return nc


def _get_program():
    global _PROGRAM
    if _PROGRAM is None:
        _PROGRAM = _build_program()
    return _PROGRAM


def _make_mask():
    m = np.zeros((B * C, B), np.float16)
    m[np.arange(B * C), np.arange(B * C) // C] = 1.0 / C
    return m


def _run_device(x, **kwargs):
    nc = _get_program()
    mask_np = _make_mask()
    in_maps = []
    for k in range(NCORES):
        xs = np.ascontiguousarray(
            x[:, :, k * HC:(k + 1) * HC, :].reshape(B * C, HC * W)
        )
        in_maps.append({"xs": xs, "mask": mask_np})
    return run_bass_kernel_spmd(nc, in_maps, list(range(NCORES)), **kwargs)


def _finalize(seg, gray):
    """seg: [B*C, S] f64 total strip sums; gray: [B, H, W] f64 channel means."""
    nodes = (seg / (H * SW)).reshape(B, C, S).transpose(0, 2, 1)      # [B,S,C]
    texture = (gray ** 2).reshape(B, H, S, SW).sum(axis=(1, 3))      # [B,S]
    feats = nodes * (1.0 - TW) + texture[..., None] * TW             # [B,S,C]

    reg = 0.0
    cur = feats
    for sz in SIZES[1:]:
        n = cur.shape[1]
        ids = np.arange(n) // 2
        counts = np.bincount(ids, minlength=sz).astype(np.float64)
        summed = np.zeros((sz, B, C), np.float64)
        np.add.at(summed, ids, cur.transpose(1, 0, 2))
        cur = (summed / counts[:, None, None]).transpose(1, 0, 2)
        reg += (cur ** 2).mean()

    diff = feats[:, :, None, :] - feats[:, None, :, :]
    d = np.sqrt((diff ** 2).sum(-1) + 1e-12)                          # [B,S,S]
    w = 1.0 - LEVEL[LCA[:S, :S]].astype(np.float64) / MAXL
    loss = (w * d ** 2 + (1.0 - w) * np.maximum(MARGIN - d, 0.0) ** 2).mean() \
        + REG_W * reg

    tree = np.broadcast_to(
        np.stack([PARENT, LEVEL], axis=-1).astype(np.int32)[None], (B, NTOT, 2)
    ).copy()
    return tree, np.float32(loss)


def _gather(res):
    seg = np.zeros((B * C, S), np.float64)
    gray = np.zeros((B, H, W), np.float64)
    for k in range(NCORES):
        seg += res.results[k]["seg_out"].astype(np.float64)
        g = res.results[k]["gray_out"].astype(np.float64)      # [B, HC*W]
        gray[:, k * HC:(k + 1) * HC, :] = g.reshape(B, HC, W)
    return seg, gray


def kernel(x):
    x = np.asarray(x, dtype=np.float32)
    res = _run_device(x)
    seg, gray = _gather(res)
    return _finalize(seg, gray)


# revision 35
# speedup vs baseline: 1.1955x; 1.0011x over previous
"""Trainium2 Bass kernel for nn_PixelAggregationNetwork.

Strategy (8 NeuronCores, memory-bound):
  x is [B=4, C=32, H=512, W=500] f32 (~131 MB). All downstream math
  (tree/LCA/loss) operates on tiny per-segment reductions of x, so the
  kernel's only real job is one streaming pass over x.

  Shard along H: core k owns rows [64k, 64k+64) for all (b, c), viewed as
  [B*C = 128 partitions, 64*500]. Input streams in 4-row chunks over THREE
  DMA paths concurrently (to saturate the ~358 GB/s per-core HBM port):
    - Pool/SWDGE: 8 chunks, casting f32->f16 in the DMA itself
    - SP-HWDGE: 4 chunks raw f32 (Pool casts them to f16 afterwards)
    - ACT-HWDGE: 4 chunks raw f32 (ACT casts them)
  Compute per chunk:
    - VectorE tensor_reduce -> per-(row,strip) sums, folded to [128, 10]
      at the end (f32 chunks are reduced straight from the f32 staging)
    - TensorE: per row, matmul with stationary [128, 4] channel-mean mask,
      moving = the row's 500 pixels f16 -> gray [4, 500] in PSUM. The 4
      rows of a chunk pack into ONE PSUM bank at base partitions
      0/32/64/96; 16 chunks over 8 banks (each bank reused once, gated by
      a standalone wait on the evacuation semaphore).
    - ACT evacuates each bank [128, 500] f32->f16 and finally writes gray
      out as 4 row-major DMAs (one per batch, partition-strided source).
  Core outputs: segment sums [128, 10] f32, gray [4, 64*500] f16.
  Host combines partials (f64) and finishes the 21-node hierarchy + loss.

  Raw Bass (no TileContext): the walrus build here supports only ONE
  embedded sync-wait per DMA/matmul/drain instruction, which Tile's
  auto-generated semaphores and kernel-tail drain violate structurally.
  With explicit semaphores, standalone single-wait EVENT_SEMAPHORE
  instructions express every multi-dependency legally.
"""

import numpy as np
from contextlib import ExitStack

import concourse.bass as bass
import concourse.mybir as mybir
from concourse.bass_utils import run_bass_kernel_spmd

B, C, H, W = 4, 32, 512, 500
S = 10
SW = W // S
TW = 0.5
MARGIN = 1.0
REG_W = 0.01

NCORES = 8
HC = H // NCORES          # 64 rows per core
R = 8                     # rows per chunk (PSUM bank group = 4 rows)
NCH = HC // R             # 8 chunks per core
NG = HC // 4              # 16 PSUM bank-groups of 4 rows
CW = R * W                # chunk free width (2000)
NBANK = 8                 # physical PSUM banks used

F32 = mybir.dt.float32
F16 = mybir.dt.float16

# chunk -> DMA path: even chunks on Pool/SWDGE (casting), odd chunks
# alternate SP / ACT (f32; Pool casts SP's, ACT casts its own).
def _path(ch):
    return "pool"

PATHS = [_path(ch) for ch in range(NCH)]
ARRIVE = []
_cnt = {"pool": 0, "sp": 0, "act": 0}
for _ch in range(NCH):
    _cnt[PATHS[_ch]] += 1
    ARRIVE.append(_cnt[PATHS[_ch]])
F32SLOT = {}
for _ch in range(NCH):
    if PATHS[_ch] != "pool":
        F32SLOT[_ch] = len(F32SLOT)
NF32 = len(F32SLOT)


# ---------------------------------------------------------------- tree/LCA
def _build_tree():
    sizes = []
    n = S
    while True:
        sizes.append(n)
        if n == 1:
            break
        n = (n + 1) // 2
    offs = np.cumsum([0] + sizes)
    total = int(offs[-1])
    parent = np.arange(total)
    level = np.zeros(total, np.int32)
    for l, sz in enumerate(sizes):
        for i in range(sz):
            g = offs[l] + i
            level[g] = l
            if l + 1 < len(sizes):
                parent[g] = offs[l + 1] + i // 2
    L = len(sizes)
    chain = np.zeros((total, L), np.int64)
    for g in range(total):
        for l in range(L):
            if l < level[g]:
                chain[g, l] = -1 - g
            else:
                a = g
                while level[a] < l:
                    a = int(parent[a])
                chain[g, l] = a
    return sizes, parent.astype(np.int32), level, chain


SIZES, PARENT, LEVEL, CHAIN = _build_tree()
MAXL = len(SIZES) - 1
NTOT = PARENT.shape[0]


def _lca_matrix():
    eq = CHAIN[:, None, :] == CHAIN[None, :, :]
    first = np.argmax(eq, axis=-1)
    return CHAIN[np.arange(NTOT)[:, None], first].astype(np.int32)


LCA = _lca_matrix()


# ---------------------------------------------------------------- device program
_PROGRAM = None


def _build_program():
    nc = bass.Bass(trn_type="TRN2", num_swdge_queues=1)
    xs = nc.declare_dram_parameter("xs", [B * C, HC * W], F32, isOutput=False)
    mask = nc.declare_dram_parameter("mask", [B * C, B], F16, isOutput=False)
    seg_out = nc.declare_dram_parameter("seg_out", [B * C, S], F32, isOutput=True)
    gray_out = nc.declare_dram_parameter("gray_out", [B, HC * W], F16, isOutput=True)

    with ExitStack() as ctx:
        t16 = ctx.enter_context(nc.sbuf_tensor([B * C, HC * W], F16))
        mask_t = ctx.enter_context(nc.sbuf_tensor([B * C, B], F16))
        segbuf = ctx.enter_context(nc.sbuf_tensor([B * C, NCH * S], F32))
        racc = ctx.enter_context(nc.sbuf_tensor([B * C, W], F16))
        seg_final = ctx.enter_context(nc.sbuf_tensor([B * C, S], F32))
        # gbuf[32q+b, (ch, w)] = gray[b, (ch*R + q)*W + w]
        gbuf = ctx.enter_context(nc.sbuf_tensor([B * C, NG * W], F16))
        psum = [ctx.enter_context(nc.psum_tensor(f"psb{i}", [B * C, W], F32))
                for i in range(NBANK)]
        a_sem = ctx.enter_context(nc.semaphore("a_sem"))    # pool in-DMAs
        b_sem = ctx.enter_context(nc.semaphore("b_sem"))    # sp in-DMAs
        pe_sem = ctx.enter_context(nc.semaphore("pe_sem"))
        ev_sem = ctx.enter_context(nc.semaphore("ev_sem"))  # evacuations
        dve_sem = ctx.enter_context(nc.semaphore("dve_sem"))
        g_sem = ctx.enter_context(nc.semaphore("g_sem"))
        s_sem = ctx.enter_context(nc.semaphore("s_sem"))

        def tsl(ch):
            return slice(ch * CW, (ch + 1) * CW)

        # ---- input DMA issues ------------------------------------------
        nc.sync.dma_start(mask_t[:], mask[:]).then_inc(b_sem, 16)
        for ch in range(NCH):
            nc.gpsimd.dma_start(t16[:, tsl(ch)], xs[:, tsl(ch)]) \
                .then_inc(a_sem, 16)

        # ---- ACT: PSUM evacuations ------------------------------------
        for e in range(NG):
            nc.scalar.wait_ge(pe_sem, e + 1)
            nc.scalar.copy(
                gbuf[:, e * W:(e + 1) * W], psum[e % NBANK][:]
            ).then_inc(ev_sem, 1)

        # ---- PE: per row, mask-stationary gray matmul -------------------
        # groups g of 4 rows; one PSUM bank per group, 16 groups over 8
        # banks (each bank reused once)
        nc.tensor.wait_ge(b_sem, 16)     # mask
        for g in range(NG):
            ch = (4 * g) // R
            if g * 4 % R == 0:
                nc.tensor.wait_ge(a_sem, 16 * (ch + 1))
            if g >= NBANK:           # bank reuse: wait for its evacuation
                nc.tensor.wait_ge(ev_sem, g - NBANK + 1)
            pst = psum[g % NBANK]
            for q in range(4):
                row = g * 4 + q
                mm = nc.tensor.matmul(
                    pst[32 * q:32 * q + B, :],
                    mask_t[:],
                    t16[:, row * W:(row + 1) * W],
                    start=True, stop=True,
                    skip_group_check=True,
                    tile_position=(0, 32 * q),
                )
            mm.then_inc(pe_sem, 1)

        # ---- ACT: gray out ----------------------------------------------
        for b in range(B):
            # gbuf partitions {b, 32+b, 64+b, 96+b} -> gray_out[b] rows
            src = gbuf[b:b + 97:32, :].rearrange(
                "q (g w) -> q g w", g=NG, w=W
            )
            dst = gray_out[b, :].rearrange(
                "(g q w) -> q g w", g=NG, q=4, w=W
            )
            nc.scalar.dma_start(dst, src).then_inc(g_sem, 16)
        nc.scalar.wait_ge(g_sem, 16 * B)

        # ---- DVE: strip sums -------------------------------------------
        # per chunk: collapse the 8 rows with fp16 pairwise adds (DVE 2x/4x
        # 16-bit modes; tensor_reduce has ~26cyc/row overhead that adds
        # avoid), then one 500-wide strip reduce into f32. fp16 partials
        # hold sums of <=8 unit-scale values: ~5e-4 relative error on a
        # term that contributes ~0.1% of the loss signal.
        with nc.allow_low_precision("fp16 row partials, see header"):
            for ch in range(NCH):
                nc.vector.wait_ge(a_sem, 16 * ARRIVE[ch])
                rows = [t16[:, (ch * R + r) * W:(ch * R + r + 1) * W]
                        for r in range(R)]
                nc.vector.tensor_add(racc[:], rows[0], rows[1])
                for r in range(2, R):
                    nc.vector.tensor_add(racc[:], racc[:], rows[r])
                nc.vector.tensor_reduce(
                    segbuf[:, ch * S:(ch + 1) * S],
                    racc[:].rearrange("p (s w) -> p s w", s=S, w=SW),
                    axis=mybir.AxisListType.X, op=mybir.AluOpType.add,
                )
        nc.vector.tensor_reduce(
            seg_final[:],
            segbuf[:].rearrange("p (c s) -> p s c", c=NCH, s=S),
            axis=mybir.AxisListType.X, op=mybir.AluOpType.add,
        ).then_inc(dve_sem, 1)

        # ---- SP: seg out ------------------------------------------------
        nc.sync.wait_ge(dve_sem, 1)
        nc.sync.dma_start(seg_out[:], seg_final[:]).then_inc(s_sem, 16)
        nc.sync.wait_ge(s_sem, 16)
    return nc


def _get_program():
    global _PROGRAM
    if _PROGRAM is None:
        _PROGRAM = _build_program()
    return _PROGRAM


def _make_mask():
    m = np.zeros((B * C, B), np.float16)
    m[np.arange(B * C), np.arange(B * C) // C] = 1.0 / C
    return m


def _run_device(x, **kwargs):
    nc = _get_program()
    mask_np = _make_mask()
    in_maps = []
    for k in range(NCORES):
        xs = np.ascontiguousarray(
            x[:, :, k * HC:(k + 1) * HC, :].reshape(B * C, HC * W)
        )
        in_maps.append({"xs": xs, "mask": mask_np})
    return run_bass_kernel_spmd(nc, in_maps, list(range(NCORES)), **kwargs)


def _finalize(seg, gray):
    """seg: [B*C, S] f64 total strip sums; gray: [B, H, W] f64 channel means."""
    nodes = (seg / (H * SW)).reshape(B, C, S).transpose(0, 2, 1)      # [B,S,C]
    texture = (gray ** 2).reshape(B, H, S, SW).sum(axis=(1, 3))      # [B,S]
    feats = nodes * (1.0 - TW) + texture[..., None] * TW             # [B,S,C]

    reg = 0.0
    cur = feats
    for sz in SIZES[1:]:
        n = cur.shape[1]
        ids = np.arange(n) // 2
        counts = np.bincount(ids, minlength=sz).astype(np.float64)
        summed = np.zeros((sz, B, C), np.float64)
        np.add.at(summed, ids, cur.transpose(1, 0, 2))
        cur = (summed / counts[:, None, None]).transpose(1, 0, 2)
        reg += (cur ** 2).mean()

    diff = feats[:, :, None, :] - feats[:, None, :, :]
    d = np.sqrt((diff ** 2).sum(-1) + 1e-12)                          # [B,S,S]
    w = 1.0 - LEVEL[LCA[:S, :S]].astype(np.float64) / MAXL
    loss = (w * d ** 2 + (1.0 - w) * np.maximum(MARGIN - d, 0.0) ** 2).mean() \
        + REG_W * reg

    tree = np.broadcast_to(
        np.stack([PARENT, LEVEL], axis=-1).astype(np.int32)[None], (B, NTOT, 2)
    ).copy()
    return tree, np.float32(loss)


def _gather(res):
    seg = np.zeros((B * C, S), np.float64)
    gray = np.zeros((B, H, W), np.float64)
    for k in range(NCORES):
        seg += res.results[k]["seg_out"].astype(np.float64)
        g = res.results[k]["gray_out"].astype(np.float64)      # [B, HC*W]
        gray[:, k * HC:(k + 1) * HC, :] = g.reshape(B, HC, W)
    return seg, gray


def kernel(x):
    x = np.asarray(x, dtype=np.float32)
    res = _run_device(x)
    seg, gray = _gather(res)
    return _finalize(seg, gray)
